# revision 47
# baseline (speedup 1.0000x reference)
"""Multi-head attention (16 heads, S=2048, d_model=1024, d_head=64) on 8 TRN2
NeuronCores, tensor-parallel over heads (2 heads per core).

Numerics: Q/K/V and the QKV projection weights ship as fp16 (e5m10, ~4.9e-4
element precision, half the DMA bytes); projection matmuls run fp16 with fp32
PSUM accumulation. Everything downstream runs float32r (fp32 storage, rne-11
matmul-input rounding, full bf16-rate on the PE). Softmax: zT is computed
transposed (sk on partitions, sq free), exp on ScalarE with the 1/sqrt(d_k)
scale fused, denominator from a ones-column appended to Vp so the P@V matmul
accumulates it for free.

Schedule (PE runs a static FIFO, so emission order is the schedule):
  phase A: per 512-chunk c, stream K,Q projections, then z+exp for every
  feasible (head, sq-group, sk-chunk), then V projection + PE-transposes,
  then the P@V accumulations. Attention for the first two sq chunks rides
  inside the DMA window this way.
  post A: remaining sq chunks run their attention with the previous chunks'
  normalize/output-projection units software-pipelined into the stream.

Host side only transposes/casts/packs inputs and sums the 8 partial outputs.
"""

import os

import numpy as np

import concourse.bass as bass
import concourse.tile as tile
from concourse import bacc, mybir
from concourse.bass_utils import run_bass_kernel_spmd

HEADS, D_K, D_V, D_X, D_M, S = 16, 64, 64, 1024, 1024, 2048
NCORES = 8
HPC = HEADS // NCORES          # heads per core
HD = HPC * D_K                 # 128: stacked head dim per core
SQW = 512                      # sq chunk width (PSUM bank = 512 fp32)
NSQ = S // SQW                 # 4
SKW = 128                      # sk chunk width (partition dim)
NSK = S // SKW                 # 16
NXC = D_X // 128               # 8 contraction chunks for projections
NJ = SQW // SKW                # 4 sk 128-chunks per 512 chunk

F32 = mybir.dt.float32
F32R = mybir.dt.float32r
F16 = mybir.dt.float16
EXP = mybir.ActivationFunctionType.Exp

LAST_EXEC_NS = None
_NC_CACHE = None


def _emit(tc, nc, aps):
    from contextlib import ExitStack

    qt, kt, vt, wq, wk, wv, wot, onescol, out = (
        aps["qt"], aps["kt"], aps["vt"], aps["wq"], aps["wk"], aps["wv"],
        aps["wot"], aps["onescol"], aps["out"],
    )

    with ExitStack() as ctx:
        wpool = ctx.enter_context(tc.tile_pool(name="weights", bufs=1))
        proj = ctx.enter_context(tc.tile_pool(name="proj", bufs=1))
        inp = ctx.enter_context(tc.tile_pool(name="inp", bufs=5))
        etp = ctx.enter_context(tc.tile_pool(name="et", bufs=42))
        outp = ctx.enter_context(tc.tile_pool(name="outs", bufs=4))
        smalls = ctx.enter_context(tc.tile_pool(name="smalls", bufs=2))
        ps_proj = ctx.enter_context(tc.tile_pool(name="ps_proj", bufs=2, space="PSUM"))
        ps_z = ctx.enter_context(tc.tile_pool(name="ps_z", bufs=2, space="PSUM"))
        ps_o = ctx.enter_context(tc.tile_pool(name="ps_o", bufs=4, space="PSUM"))

        # ---- persistent SBUF tensors ----
        wq_sb = wpool.tile([128, D_X], F16, tag="wq")     # (xc p) stacked chunks
        wk_sb = wpool.tile([128, D_X], F16, tag="wk")
        wv_sb = wpool.tile([128, D_X], F16, tag="wv")
        wot_sb = wpool.tile([HD, D_M], F32R, tag="wot")
        qpt_sb = proj.tile([HD, S], F32R, tag="qpt")
        kpt_sb = proj.tile([HD, S], F32R, tag="kpt")
        # VpAug: per (h, skc) a (128 sk, 65) block: cols 0-63 = Vp, col 64 = 1
        vpa_sb = proj.tile([128, HPC * NSK * 65], F32R, tag="vpa")
        headst_sb = proj.tile([HD, S], F32R, tag="headst")

        # weight DMAs are interleaved with the first chunk loads so the
        # K/Q data the first attention steps need is in flight ASAP.
        def load_w(w_dram, w_sb):
            nc.sync.dma_start(
                w_sb[:].rearrange("p (c f) -> p c f", f=128),
                w_dram.rearrange("(c p) f -> p c f", p=128),
            )

        def load_aux():
            onescol_r = onescol.bitcast(F32R)
            # ones column of VpAug (col 64 of each 65-wide block)
            nc.sync.dma_start(
                vpa_sb[:].rearrange("p (c f) -> p c f", f=65)[:, :, 64:65],
                onescol_r[:, 0:HPC * NSK].rearrange("p (c o) -> p c o", o=1),
            )
            nc.sync.dma_start(wot_sb[:], wot.bitcast(F32R))

        def load_chunk(tt_dram, c, name):
            """One DMA: all 8 xc strips of a 512-wide chunk -> (128, 8, 512)."""
            t = inp.tile([128, NXC, SQW], F16, tag="inp", name=name)
            nc.sync.dma_start(
                t[:],
                tt_dram.rearrange("(xc p) s -> p xc s", p=128)[
                    :, :, c * SQW:(c + 1) * SQW
                ],
            )
            return t

        def project(t, w_sb, dst_sb, c):
            """Compute dst_sb[:, c*512:(c+1)*512] = W.T @ X.T chunk (fp16)."""
            ps = ps_proj.tile([128, SQW], F32, tag="ps_proj")
            for xc in range(NXC):
                nc.tensor.matmul(
                    ps[:],
                    w_sb[:, xc * 128:(xc + 1) * 128],
                    t[:, xc, :],
                    start=(xc == 0),
                    stop=(xc == NXC - 1),
                )
            nc.vector.tensor_copy(dst_sb[:, c * SQW:(c + 1) * SQW], ps[:])

        def project_v(t, c):
            """VpAug sk-chunks for 512-chunk c: Vp = VT_chunk.T @ Wv directly
            in (sk, hd) layout (fp16, N=128 runs at full rate)."""
            for j in range(NJ):
                skc = c * NJ + j
                ps = ps_proj.tile([128, HD], F32, tag="ps_proj", name=f"vp_{skc}")
                for xc in range(NXC):
                    nc.tensor.matmul(
                        ps[:],
                        t[:, xc, j * SKW:(j + 1) * SKW],
                        wv_sb[:, xc * 128:(xc + 1) * 128],
                        start=(xc == 0),
                        stop=(xc == NXC - 1),
                    )
                for h in range(HPC):
                    base = (h * NSK + skc) * 65
                    nc.vector.tensor_copy(
                        vpa_sb[:, base:base + 64],
                        ps[:, h * 64:(h + 1) * 64],
                    )

        def z_exp(h, sqc, skc):
            """z matmul + exp; returns the ET tile for the AV step."""
            z_ps = ps_z.tile([128, SQW], F32, tag="ps_z")
            nc.tensor.matmul(
                z_ps[:],
                kpt_sb[h * 64:(h + 1) * 64, skc * SKW:(skc + 1) * SKW],
                qpt_sb[h * 64:(h + 1) * 64, sqc * SQW:(sqc + 1) * SQW],
                start=True,
                stop=True,
            )
            et = etp.tile([128, SQW], F32R, tag="et")
            nc.scalar.activation(et[:], z_ps[:], EXP, scale=1.0 / 8.0)
            return et

        def av(h, skc, et, o_ps):
            base = (h * NSK + skc) * 65
            nc.tensor.matmul(
                o_ps[:],
                vpa_sb[:, base:base + 65],
                et[:],
                start=(skc == 0),
                stop=(skc == NSK - 1),
            )

        def normalize(h, sqc, o_ps, copy_engine="vector"):
            """heads_h[:, sqc chunk] = o[0:64] / o[64]. The reciprocal row is
            replicated across partitions on the idle GpSimd engine."""
            rec = smalls.tile([1, SQW], F32, tag="rec")
            nc.vector.reciprocal(rec[:], o_ps[64:65, :])
            rec64 = smalls.tile([64, SQW], F32, tag="rec64")
            nc.gpsimd.partition_broadcast(rec64[:], rec[:])
            nc.vector.tensor_mul(
                headst_sb[h * 64:(h + 1) * 64, sqc * SQW:(sqc + 1) * SQW],
                o_ps[0:64, :],
                rec64[:],
            )

        def outproj_unit(sqt, copy_engine="vector"):
            """One out[128 x 1024] row-tile: 2 matmuls, 2 copies, 1 DMA."""
            ot = outp.tile([128, D_M], F32, tag="ot")
            for dmc in range(D_M // SQW):
                op = ps_proj.tile([128, SQW], F32, tag="ps_proj")
                nc.tensor.matmul(
                    op[:],
                    headst_sb[:, sqt * 128:(sqt + 1) * 128],
                    wot_sb[:, dmc * SQW:(dmc + 1) * SQW],
                    start=True,
                    stop=True,
                )
                dst = ot[:, dmc * SQW:(dmc + 1) * SQW]
                if copy_engine == "scalar":
                    nc.scalar.copy(dst, op[:])
                else:
                    nc.vector.tensor_copy(dst, op[:])
            nc.sync.dma_start(out[sqt * 128:(sqt + 1) * 128, :], ot[:])

        def outproj_units(sqc):
            return [
                (lambda copy_engine="vector", sqt=sqc * (SQW // 128) + j:
                 outproj_unit(sqt, copy_engine))
                for j in range(SQW // 128)
            ]

        # ---- phase A: stream projections. Attention z/exp is emitted for sq
        # chunks 0..2 as soon as their K/Q chunks land (PE FIFO never blocks
        # on a pending AV); AVs accumulate for chunks 0,1 (4 PSUM banks),
        # while chunk 2's ET tiles are stored in SBUF for post-A AVs. ----
        NG0 = 2       # groups accumulated during phase A
        NGE = 3       # groups whose z/exp runs during phase A
        oa = {}
        ets2 = {}     # (h, skc) -> stored ET tile for sq chunk 2
        for c in range(NSQ):
            if c == 0:
                tk = load_chunk(kt, c, f"kc_{c}")
                load_w(wk, wk_sb)
                tq = load_chunk(qt, c, f"qc_{c}")
                load_w(wq, wq_sb)
                tv = load_chunk(vt, c, f"vc_{c}")
                load_w(wv, wv_sb)
                load_aux()
            else:
                tk = load_chunk(kt, c, f"kc_{c}")
                tq = load_chunk(qt, c, f"qc_{c}")
                tv = load_chunk(vt, c, f"vc_{c}")
            project(tk, wk_sb, kpt_sb, c)
            project(tq, wq_sb, qpt_sb, c)
            steps = []     # (h, g, skc) feasible this round
            for g in range(NGE):
                if g > c:
                    continue
                lo = 0 if g == c else c * NJ
                for skc in range(lo, (c + 1) * NJ):
                    for h in range(HPC):
                        steps.append((h, g, skc))
            ets = [z_exp(h, g, skc) for (h, g, skc) in steps]
            project_v(tv, c)
            for (h, g, skc), et in zip(steps, ets):
                if g >= NG0:
                    ets2[(h, skc)] = et
                    continue
                if (h, g) not in oa:
                    oa[(h, g)] = ps_o.tile([65, SQW], F32, tag="ps_o",
                                           name=f"oa_{h}_{g}")
                av(h, skc, et, oa[(h, g)])
            if c == NSQ - 1:
                # chunks 0/1 are complete: normalize them now, freeing their
                # PSUM banks so chunk 2's stored AVs run before post-A
                for g in range(NG0):
                    for h in range(HPC):
                        normalize(h, g, oa[(h, g)])
                ob2 = {h: ps_o.tile([65, SQW], F32, tag="ps_o",
                                    name=f"ob2_{h}")
                       for h in range(HPC)}
                for skc in range(NSK):
                    for h in range(HPC):
                        av(h, skc, ets2.pop((h, skc)), ob2[h])
                for h in range(HPC):
                    normalize(h, 2, ob2[h])

        # ---- post A: only chunk 3's attention remains; chunks 0/1/2's
        # outproj units interleave into its stream as PE fillers ----
        fillers = outproj_units(0) + outproj_units(1) + outproj_units(2)
        ob3 = {h: ps_o.tile([65, SQW], F32, tag="ps_o", name=f"ob3_{h}")
               for h in range(HPC)}
        fi = 0
        pend = []   # chunk-3 AVs trail its z/exp stream by one skc
        for skc in range(NSK):
            nxt = [(h, skc, z_exp(h, 3, skc)) for h in range(HPC)]
            for h, pskc, et in pend:
                av(h, pskc, et, ob3[h])
            pend = nxt
            if fi < len(fillers):
                fillers[fi]()
                fi += 1
        for h, pskc, et in pend:
            av(h, pskc, et, ob3[h])
        while fi < len(fillers):
            fillers[fi]()
            fi += 1
        for h in range(HPC):
            normalize(h, 3, ob3[h], copy_engine="scalar")
        for i, f in enumerate(outproj_units(3)):
            f(copy_engine="scalar" if i % 2 else "vector")


def _build_nc():
    nc = bacc.Bacc("TRN2", target_bir_lowering=False, debug=False,
                   num_devices=NCORES)
    aps = {
        "qt": nc.dram_tensor("qt", [D_X, S], F16, kind="ExternalInput").ap(),
        "kt": nc.dram_tensor("kt", [D_X, S], F16, kind="ExternalInput").ap(),
        "vt": nc.dram_tensor("vt", [D_X, S], F16, kind="ExternalInput").ap(),
        "wq": nc.dram_tensor("wq", [D_X, HD], F16, kind="ExternalInput").ap(),
        "wk": nc.dram_tensor("wk", [D_X, HD], F16, kind="ExternalInput").ap(),
        "wv": nc.dram_tensor("wv", [D_X, HD], F16, kind="ExternalInput").ap(),
        "wot": nc.dram_tensor("wot", [HD, D_M], F32, kind="ExternalInput").ap(),
        "onescol": nc.dram_tensor("onescol", [128, 64], F32, kind="ExternalInput").ap(),
        "out": nc.dram_tensor("out", [S, D_M], F32, kind="ExternalOutput").ap(),
    }
    with tile.TileContext(nc) as tc:
        with nc.allow_low_precision(reason="f32r/fp16 matmul pipeline"):
            _emit(tc, nc, aps)
    nc.compile()
    return nc


def kernel(**inputs):
    global LAST_EXEC_NS, _NC_CACHE
    Q = np.asarray(inputs["Q"], dtype=np.float32)
    K = np.asarray(inputs["K"], dtype=np.float32)
    V = np.asarray(inputs["V"], dtype=np.float32)
    W_q = np.asarray(inputs["W_q"], dtype=np.float32)
    W_k = np.asarray(inputs["W_k"], dtype=np.float32)
    W_v = np.asarray(inputs["W_v"], dtype=np.float32)
    W_o = np.asarray(inputs["W_o"], dtype=np.float32)

    QT = np.ascontiguousarray(Q.T.astype(np.float16))
    KT = np.ascontiguousarray(K.T.astype(np.float16))
    VT = np.ascontiguousarray(V.T.astype(np.float16))
    in_maps = []
    for c in range(NCORES):
        h0 = HPC * c
        in_maps.append({
            "qt": QT, "kt": KT, "vt": VT,
            "wq": np.ascontiguousarray(np.concatenate(
                [W_q[h0 + i] for i in range(HPC)], axis=1).astype(np.float16)),
            "wk": np.ascontiguousarray(np.concatenate(
                [W_k[h0 + i] for i in range(HPC)], axis=1).astype(np.float16)),
            "wv": np.ascontiguousarray(np.concatenate(
                [W_v[h0 + i] for i in range(HPC)], axis=1).astype(np.float16)),
            "wot": np.ascontiguousarray(W_o[:, c * HD:(c + 1) * HD].T),
            "onescol": np.ones((128, 64), np.float32),
        })

    if _NC_CACHE is None:
        _NC_CACHE = _build_nc()
    nc = _NC_CACHE

    trace = bool(os.environ.get("MHA_TRACE"))
    res = None
    if trace:
        try:
            res = run_bass_kernel_spmd(nc, in_maps, list(range(NCORES)),
                                       trace=True)
        except Exception as e:  # profiling infra unavailable -> run untraced
            print(f"[kernel] traced run failed ({e!r}); falling back")
            res = None
    if res is None:
        res = run_bass_kernel_spmd(nc, in_maps, list(range(NCORES)))

    LAST_EXEC_NS = getattr(res, "exec_time_ns", None)

    out = np.zeros((S, D_M), np.float32)
    for r in res.results:
        out += r["out"]
    return out
